# revision 1
# baseline (speedup 1.0000x reference)
"""Multi-head attention block (b=8, n=1024, d=1024, heads=16) on 8 trn2
NeuronCores, data-parallel over batch (one batch element per core).

Matmul operands are bf16 (PE streams 1 col/cycle; fp32 is 4 cycles/col,
fp32r ~2); PSUM accumulation and all softmax math stay fp32. End-to-end
absmax error vs the fp32 reference is ~3e-3 of scale.

Per-core dataflow (all matmuls on PE):
  B:  qkT[c, t]  = sum_d WqkvT[d, c] * xT[d, t]      (q,k channels 0..2047)
  C:  V[t, c]    = sum_d xT[d, t]    * WqkvT[d, 2048+c]
  D:  per HEAD PAIR (the two K=64 S^T matmuls run concurrently on PE row
      groups 0-63 / 64-127, into the two banks of a shared [128,1024] PSUM
      tile, so one exp covers both heads and the S^T wall halves):
        S^T[j, i] = sum_d kT[d, j] qT[d, i]           (K=64 matmul)
        E = exp(S^T * scale)                          (ACT, no max-subtract:
                                                       |scores*scale| < ~3)
        [O^T_u; rowsum] = [V_h | 1]^T E               (ones column appended to
                                                       V gives rowsum for free)
        O^T = O^T_u * (1/rowsum broadcast)            (1/x = exp(-ln x) on ACT
                                                       -- shares the Exp table;
                                                       broadcast via K=1 PE
                                                       outer product)
  E:  yT[o, t] = sum_D WprojT[D, o] O^T[D, t] + bias[o]

Overlap structure: stage C is woven with pair 0's S^T/exp stream so ACT
starts early; each B tile-pair is emitted one head-pair ahead of the heads
that consume it; each pair's AV matmuls are woven one j-step behind its
S^T stream; normalization broadcasts run after the next pair's B matmuls
so the ACT reciprocal chain never stalls the PE queue.

Layout trick: softmax normalization needs a per-column scale on O^T_u; the
reciprocal row sits on PSUM partition 64, is broadcast to [64, 512] with a
K=1 matmul, then one DVE multiply normalizes. Odd heads land on SBUF
partitions 64..127 of the O^T tile via a SBUF->SBUF DMA (DVE lanes are
partition-local and cannot shift partitions).

Host does only data movement: transposes / tiling rearranges of x and the
weights (cast to bf16), and the inverse transpose of the output.
"""

import json

import ml_dtypes
import numpy as np

D = 1024
NT = 1024
H = 16
HD = 64
P = 128
DC = D // P  # 8 contraction chunks
SCALE = HD ** -0.5
N_CORES = 8

_CACHE = {}


# --------------------------------------------------------------------------
# Workaround for the walrus build in this container: each TPB instruction
# encodes at most ONE sync wait (NEURON_ISA_TPB_EVENTS has a single wait
# slot) and this walrus version errors out instead of splitting. Tile
# attaches several waits per instruction. Hoist all but the last wait onto
# preceding single-wait EventSemaphore no-ops on the same (in-order) engine.
# --------------------------------------------------------------------------
def _split_sync_waits_json(bir_bytes: bytes) -> bytes:
    j = json.loads(bir_bytes)
    changed = False
    ctr = 0
    dma_ops = {"TensorLoad", "TensorSave", "TensorCopy", "TensorReduce"}
    for fn in j.get("functions", []):
        for blk in fn.get("blocks", []):
            out = []
            for inst in blk.get("instructions", []):
                si = inst.get("sync_info")
                if si:
                    waits = si.get("on_wait") or []
                    if len(waits) > 1:
                        for w in waits[:-1]:
                            ctr += 1
                            out.append(
                                {
                                    "debug": inst.get("debug", 0),
                                    "engine": inst.get("engine"),
                                    "ins": [],
                                    "outs": [],
                                    "name": f"splitw-{ctr}-{inst['name']}",
                                    "opcode": "EventSemaphore",
                                    "sync_info": {"on_update": [], "on_wait": [w]},
                                }
                            )
                        si["on_wait"] = [waits[-1]]
                        changed = True
                    ups = si.get("on_update") or []
                    if len(ups) > 1 and inst.get("opcode") not in dma_ops:
                        extra = ups[:-1]
                        si["on_update"] = [ups[-1]]
                        out.append(inst)
                        for u in extra:
                            ctr += 1
                            out.append(
                                {
                                    "debug": inst.get("debug", 0),
                                    "engine": inst.get("engine"),
                                    "ins": [],
                                    "outs": [],
                                    "name": f"splitu-{ctr}-{inst['name']}",
                                    "opcode": "EventSemaphore",
                                    "sync_info": {"on_update": [u], "on_wait": []},
                                }
                            )
                        changed = True
                        continue
                out.append(inst)
            blk["instructions"] = out
    if not changed:
        return bir_bytes
    return json.dumps(j).encode()


def _install_bir_fix():
    import concourse.bass as bass

    if getattr(bass.Bass, "_split_waits_patched", False):
        return
    orig = bass.Bass.to_json_bytes

    def patched(self, *a, **kw):
        return _split_sync_waits_json(orig(self, *a, **kw))

    bass.Bass.to_json_bytes = patched
    bass.Bass._split_waits_patched = True


def _build_module():
    from contextlib import ExitStack

    import concourse.bass as bass
    import concourse.tile as tile
    from concourse import mybir

    _install_bir_fix()
    f32 = mybir.dt.float32
    # bf16 matmul operands: PE streams 1 col/cycle at 2.4 GHz (fp32 is 4
    # cycles/col, fp32r ~2). PSUM accumulation and all softmax math stay
    # fp32; end-to-end absmax error vs the fp32 reference is ~3e-3 of scale.
    bf16 = mybir.dt.bfloat16
    nc = bass.Bass(num_swdge_queues=4)

    xT = nc.declare_dram_parameter("xT", [D, NT], bf16, isOutput=False)
    # wqk[p, ct, a, c] = W_qkv.T[a*128+p, ct*128+c]  (q,k channels, ct<16)
    wqk = nc.declare_dram_parameter("wqk", [P, 16, DC, P], bf16, isOutput=False)
    # wv[p, a, cv] = W_qkv.T[a*128+p, 2048+cv]
    wvp = nc.declare_dram_parameter("wv", [P, DC, D], bf16, isOutput=False)
    # wpr[p, ot, a, c] = W_proj.T[a*128+p, ot*128+c]
    wpr = nc.declare_dram_parameter("wpr", [P, DC, DC, P], bf16, isOutput=False)
    # biasT[p, t] = b_proj[t*128+p]
    biasT = nc.declare_dram_parameter("biasT", [P, DC], f32, isOutput=False)
    yT = nc.declare_dram_parameter("yT", [D, NT], f32, isOutput=True)

    with tile.TileContext(nc) as tc, ExitStack() as outer:
        v_pool = outer.enter_context(tc.tile_pool(name="vsb", bufs=1))
        ot_pool = outer.enter_context(tc.tile_pool(name="otp", bufs=1))
        qk_pool = outer.enter_context(tc.tile_pool(name="qkp", bufs=4))
        misc = outer.enter_context(tc.tile_pool(name="misc", bufs=1))
        xt_pool = outer.enter_context(tc.tile_pool(name="xt", bufs=1))
        wt_pool = outer.enter_context(tc.tile_pool(name="wt", bufs=5))

        v_sb = v_pool.tile([P, DC, H, HD + 1], bf16)  # V + ones column per head
        ot = ot_pool.tile([P, DC, NT], bf16)          # O^T, channel-major
        ones_f = misc.tile([P, HD], f32)
        ones_t = misc.tile([P, HD], bf16)
        bias_t = misc.tile([P, DC], f32)
        nc.vector.memset(ones_f[:], 1.0)
        nc.vector.tensor_copy(ones_t[:], ones_f[:])
        nc.gpsimd.dma_start(bias_t[:], biasT[:])
        for vt in range(DC):
            nc.vector.tensor_copy(v_sb[:, vt, :, HD], ones_f[:, 0:H])

        xt = xt_pool.tile([P, DC, NT], bf16)
        wt0 = wt_pool.tile([P, DC, P], bf16, tag="wt")
        nc.gpsimd.dma_start(wt0[:], wqk[:, 0, :, :])
        wt8 = wt_pool.tile([P, DC, P], bf16, tag="wt")
        nc.gpsimd.dma_start(wt8[:], wqk[:, 8, :, :])
        for a in range(DC):
            nc.gpsimd.dma_start(xt[:, a, :], xT[a * P : (a + 1) * P, :])

        # ------- stages B+D interleaved: qk projection + attention -------
        # B tile-pairs are emitted one head-pair ahead of the heads that
        # consume them; each head's AV matmuls are woven between its own
        # S^T matmuls (2 behind) so the PE never drains while ACT works
        # through the exps. Each head's softmax normalization is split:
        # the ACT part (ln/exp) runs at the START of the next head's block
        # (ahead of its 8 exps in the ACT queue), the PE/DVE part at the
        # END of the next block. This keeps the PE dense enough for the
        # HAM clock gate to hold 2.4 GHz.
        with (
            tc.tile_pool(name="es", bufs=18) as es_pool,
            tc.tile_pool(name="tmp", bufs=4) as tmp_pool,
            tc.tile_pool(name="rsp", bufs=5) as rs_pool,
            tc.tile_pool(name="rbp", bufs=4) as rb_pool,
            tc.tile_pool(name="psS", bufs=2, space="PSUM") as psS,
            tc.tile_pool(name="psO", bufs=1, space="PSUM") as psO,
        ):

            def emit_b(ct, wt=None):
                # qkT[c, t] for one 128-channel tile (2 heads' q or k)
                if wt is None:
                    wt = wt_pool.tile([P, DC, P], bf16, tag="wt")
                    nc.gpsimd.dma_start(wt[:], wqk[:, ct, :, :])
                ps = psS.tile([P, NT], f32, tag="sps")
                for a in range(DC):
                    for nh in range(2):
                        nc.tensor.matmul(
                            ps[:, nh * 512 : (nh + 1) * 512],
                            wt[:, a, :],
                            xt[:, a, nh * 512 : (nh + 1) * 512],
                            start=(a == 0),
                            stop=(a == DC - 1),
                        )
                if ct < 8:
                    t = qk_pool.tile([P, NT], bf16, tag="qt")
                else:
                    t = qk_pool.tile([P, NT], bf16, tag="kt")
                # copy on ACT, not DVE: at the pair boundary the DVE queue
                # is deep in norm work while ACT sits in its exp bubble;
                # this frees the borrowed psS slot sooner for the next
                # pair's S^T matmuls
                nc.scalar.copy(t[:], ps[:])
                return t

            def act_recip(out, in_):
                # ACT-table reciprocal. bass's activation() refuses
                # Reciprocal for accuracy reasons, but rowsum is in
                # [n, n*e^3] and the softmax weights are bf16 anyway;
                # measured end-to-end impact is below the bf16 noise.
                eng = nc.scalar
                inputs = [eng.lower_ap(in_)]
                for arg in (0.0, 1.0, 0.0):  # bias, scale, alpha
                    inputs.append(
                        mybir.ImmediateValue(dtype=f32, value=arg)
                    )
                return eng.add_instruction(
                    mybir.InstActivation(
                        name=nc.get_next_instruction_name(),
                        func=mybir.ActivationFunctionType.Reciprocal,
                        ins=inputs,
                        outs=[eng.lower_ap(out)],
                    )
                )

            def norm_full(h, opx0, opx1):
                # inline softmax normalization for one head (recip already
                # queued on ACT): broadcast 1/rowsum via K=1 outer product
                # into a free psS slot, then one DVE multiply per half.
                odd = h % 2 == 1
                if odd:
                    tmp = tmp_pool.tile([HD, NT], bf16)
                else:
                    tmp = None
                bps = psS.tile([HD, NT], f32, tag="sps")
                for ih, ops, rsr in ((0,) + opx0, (1,) + opx1):
                    nc.tensor.matmul(
                        bps[:, ih * 512 : (ih + 1) * 512],
                        ones_t[HD : HD + 1, :],
                        rsr[HD : HD + 1, :], start=True, stop=True,
                    )
                rb = rb_pool.tile([HD, NT], f32)
                nc.vector.tensor_copy(rb[:], bps[:])
                for ih, ops, rsr in ((0,) + opx0, (1,) + opx1):
                    dst = (
                        tmp[:, ih * 512 : (ih + 1) * 512]
                        if odd
                        else ot[0:HD, h // 2, ih * 512 : (ih + 1) * 512]
                    )
                    nc.vector.tensor_mul(
                        dst, ops[0:HD, :], rb[:, ih * 512 : (ih + 1) * 512]
                    )
                if odd:
                    # DVE lanes cannot shift partitions; DMA moves the odd
                    # head's rows to partitions 64..127
                    nc.gpsimd.dma_start(ot[HD:P, h // 2, :], tmp[:])

            # process heads in PAIRS: the two heads' K=64 S^T matmuls run
            # CONCURRENTLY on PE row groups 0-63 / 64-127 (row tiling), into
            # the two banks of a shared [P, 1024] PSUM tile, so one exp
            # covers both heads and the S^T wall halves.
            def pair_block(hp, qt, kt, es_pre=None):
                hA, hB = 2 * hp, 2 * hp + 1
                qsA, ksA = qt[0:HD, :], kt[0:HD, :]
                qsB, ksB = qt[HD:P, :], kt[HD:P, :]
                es_list = [None] * DC  # es_list[j] = (es_ih0, es_ih1)
                opA = opB = None

                def emit_st(j):
                    out = []
                    for ih in range(2):
                        sps = psS.tile([P, NT], f32, tag="sps")
                        for qs, ks, half in ((qsA, ksA, 0), (qsB, ksB, 1)):
                            nc.tensor.matmul(
                                sps[:, half * 512 : (half + 1) * 512],
                                ks[:, j * P : (j + 1) * P],
                                qs[:, ih * 512 : (ih + 1) * 512],
                                start=True,
                                stop=True,
                            )
                        es = es_pool.tile([P, NT], bf16)
                        nc.scalar.activation(
                            es[:], sps[:], mybir.ActivationFunctionType.Exp,
                            scale=SCALE,
                        )
                        out.append(es)
                    return tuple(out)

                def do_av(j):
                    for ih in range(2):
                        for half, h, ops in ((0, hA, opA), (1, hB, opB)):
                            nc.tensor.matmul(
                                ops[ih][0 : HD + 1, :],
                                v_sb[:, j, h, :],
                                es_list[j][ih][:, half * 512 : (half + 1) * 512],
                                start=(j == 0),
                                stop=(j == DC - 1),
                            )

                for j in range(DC):
                    es_list[j] = es_pre[j] if es_pre is not None else emit_st(j)
                    if j >= 1:
                        if j == 1:
                            opA0 = psO.tile([P, 512], f32, tag="opA0")
                            opA1 = psO.tile([P, 512], f32, tag="opA1")
                            opB0 = psO.tile([P, 512], f32, tag="opB0")
                            opB1 = psO.tile([P, 512], f32, tag="opB1")
                            opA = (opA0, opA1)
                            opB = (opB0, opB1)
                        do_av(j - 1)
                do_av(DC - 1)
                # reciprocals queue on ACT right behind this pair's exps;
                # the PE-side normalization runs after the next pair's B
                # matmuls so the recip latency is hidden
                rsA0 = norm_recip(opA[0])
                rsA1 = norm_recip(opA[1])
                rsB0 = norm_recip(opB[0])
                rsB1 = norm_recip(opB[1])
                rsA = (rsA0, rsA1)
                rsB = (rsB0, rsB1)
                return (
                    (hA, (opA[0], rsA[0]), (opA[1], rsA[1])),
                    (hB, (opB[0], rsB[0]), (opB[1], rsB[1])),
                )

            def norm_recip_dve(ops):
                # DVE reciprocal is slow per element (one lane) but the DVE
                # has slack; splitting the four per-pair reciprocals 2/2
                # between ACT and DVE keeps both off the critical path
                rs = rs_pool.tile([P, 512], f32)
                nc.vector.reciprocal(rs[HD : HD + 1, :], ops[HD : HD + 1, :])
                rsr = rs_pool.tile([P, 512], bf16)
                nc.vector.tensor_copy(rsr[HD : HD + 1, :], rs[HD : HD + 1, :])
                return rsr

            def norm_recip(ops):
                # 1/rowsum = exp(-ln(rowsum)): Ln and Exp share an ACT
                # table set; the Reciprocal table costs a ~1.5us
                # ACT_TABLE_LOAD per switch and slows every ACTIVATE
                rs = rs_pool.tile([P, 512], f32)
                nc.scalar.activation(
                    rs[HD : HD + 1, :], ops[HD : HD + 1, :],
                    mybir.ActivationFunctionType.Ln,
                )
                rsr = rs_pool.tile([P, 512], bf16)
                nc.scalar.activation(
                    rsr[HD : HD + 1, :], rs[HD : HD + 1, :],
                    mybir.ActivationFunctionType.Exp, scale=-1.0,
                )
                return rsr

            # ---- stage C (V = x @ Wv^T), woven with pair 0's S^T/exp ----
            qt = emit_b(0, wt=wt0)
            kt = emit_b(8, wt=wt8)
            es0 = [None] * DC
            with tc.tile_pool(name="wvt", bufs=1) as wv_pool:
                wv = wv_pool.tile([P, DC, D], bf16)
                nc.gpsimd.dma_start(wv[:], wvp[:])
                for vt in range(DC):
                    # alternate across all four psO tags so consecutive vt
                    # iterations double-buffer (each tag has bufs=1)
                    if vt % 2 == 0:
                        pv0 = psO.tile([P, 512], f32, tag="opA0")
                        pv1 = psO.tile([P, 512], f32, tag="opA1")
                    else:
                        pv0 = psO.tile([P, 512], f32, tag="opB0")
                        pv1 = psO.tile([P, 512], f32, tag="opB1")
                    for a in range(DC):
                        for ch, ps in ((0, pv0), (1, pv1)):
                            nc.tensor.matmul(
                                ps[:],
                                xt[:, a, vt * P : (vt + 1) * P],
                                wv[:, a, ch * 512 : (ch + 1) * 512],
                                start=(a == 0),
                                stop=(a == DC - 1),
                            )
                    # weave pair 0's S^T so ACT starts its exps early
                    j = vt
                    for ih in range(2):
                        sps = psS.tile([P, NT], f32, tag="sps")
                        for qo2 in (0, HD):
                            nc.tensor.matmul(
                                sps[:, (qo2 // HD) * 512 : (qo2 // HD + 1) * 512],
                                kt[qo2 : qo2 + HD, j * P : (j + 1) * P],
                                qt[qo2 : qo2 + HD, ih * 512 : (ih + 1) * 512],
                                start=True,
                                stop=True,
                            )
                        es = es_pool.tile([P, NT], bf16)
                        nc.scalar.activation(
                            es[:], sps[:], mybir.ActivationFunctionType.Exp,
                            scale=SCALE,
                        )
                        if es0[j] is None:
                            es0[j] = [None, None]
                        es0[j][ih] = es
                    for ch, ps in ((0, pv0), (1, pv1)):
                        # one strided copy per half (dst skips each head's
                        # ones column) instead of 8 small copies: same
                        # bytes, 1/8th the DVE instruction overhead
                        nc.vector.tensor_copy(
                            v_sb[:, vt, ch * 8 : (ch + 1) * 8, 0:HD],
                            ps[:].rearrange("p (h d) -> p h d", h=8),
                        )
            es0 = [tuple(e) for e in es0]

            for hp in range(8):
                res = pair_block(hp, qt, kt, es_pre=es0 if hp == 0 else None)
                if hp + 1 < 8:
                    qt = emit_b(hp + 1)
                    kt = emit_b(8 + hp + 1)
                for entry in res:
                    norm_full(*entry)

        # -------- stage E: output projection + bias --------
        with (
            tc.tile_pool(name="wp", bufs=1) as wp_pool,
            tc.tile_pool(name="outp", bufs=3) as out_pool,
            tc.tile_pool(name="psE", bufs=2, space="PSUM") as psE,
        ):
            # prefetch the whole 2 MB of proj weights up front; the DMAs
            # overlap the tail of the attention phase
            wpt_all = wp_pool.tile([P, DC, DC, P], bf16)
            for oi in range(DC):
                nc.gpsimd.dma_start(wpt_all[:, oi, :, :], wpr[:, oi, :, :])
            for oi in range(DC):
                wpt = wpt_all[:, oi, :, :]
                osb = out_pool.tile([P, NT], f32)
                pe = psE.tile([P, NT], f32, tag="pse")
                for a in range(DC):
                    for nh in range(2):
                        nc.tensor.matmul(
                            pe[:, nh * 512 : (nh + 1) * 512],
                            wpt[:, a, :],
                            ot[:, a, nh * 512 : (nh + 1) * 512],
                            start=(a == 0),
                            stop=(a == DC - 1),
                        )
                nc.vector.tensor_scalar_add(osb[:], pe[:], bias_t[:, oi : oi + 1])
                nc.gpsimd.dma_start(yT[oi * P : (oi + 1) * P, :], osb[:])

    return nc


def _get_nc():
    if "nc" not in _CACHE:
        _CACHE["nc"] = _build_module()
    return _CACHE["nc"]


def _host_inputs(x, W_qkv, W_proj, b_proj):
    bf = ml_dtypes.bfloat16
    x = np.asarray(x, dtype=np.float32).astype(bf)
    W_qkv = np.asarray(W_qkv, dtype=np.float32).astype(bf)
    W_proj = np.asarray(W_proj, dtype=np.float32).astype(bf)
    b_proj = np.asarray(b_proj, dtype=np.float32)

    wqkvT = W_qkv.T  # [1024, 3072]
    # wqk[p, ct, a, c] = wqkvT[a*128+p, ct*128+c] for q,k channels
    wqk = np.ascontiguousarray(
        wqkvT[:, : 2 * D].reshape(DC, P, 16, P).transpose(1, 2, 0, 3)
    )
    # wv[p, a, cv] = wqkvT[a*128+p, 2048+cv]
    wv = np.ascontiguousarray(wqkvT[:, 2 * D :].reshape(DC, P, D).transpose(1, 0, 2))
    # wpr[p, ot, a, c] = W_proj.T[a*128+p, ot*128+c]
    wpr = np.ascontiguousarray(
        W_proj.T.reshape(DC, P, DC, P).transpose(1, 2, 0, 3)
    )
    biasT = np.ascontiguousarray(b_proj.reshape(DC, P).T)

    in_maps = []
    for i in range(N_CORES):
        in_maps.append(
            {
                "xT": np.ascontiguousarray(x[i].T),
                "wqk": wqk,
                "wv": wv,
                "wpr": wpr,
                "biasT": biasT,
            }
        )
    return in_maps


def _run(in_maps, trace=False):
    from concourse.bass_utils import run_bass_kernel_spmd

    nc = _get_nc()
    return run_bass_kernel_spmd(nc, in_maps, list(range(N_CORES)), trace=trace)


def kernel(x, W_qkv, W_proj, b_proj):
    in_maps = _host_inputs(x, W_qkv, W_proj, b_proj)
    res = _run(in_maps)
    out = np.stack([res.results[i]["yT"].T for i in range(N_CORES)], axis=0)
    return np.ascontiguousarray(out, dtype=np.float32)



# revision 5
# speedup vs baseline: 23083.3485x; 23083.3485x over previous
"""Multi-head attention block (b=8, n=1024, d=1024, heads=16) on 8 trn2
NeuronCores, data-parallel over batch (one batch element per core).

Matmul operands are bf16/fp16 (PE streams 1 col/cycle; fp32 is 4 cycles/col);
PSUM accumulation and softmax reciprocal stay fp32.

Per-core dataflow (all matmuls on PE):
  B:  qkT[c, t]  = sum_d WqkvT[d, c] * xT[d, t]      (q,k channels 0..2047)
  C:  V[t, c]    = sum_d xT[d, t]    * WqkvT[d, 2048+c]
  D:  per HEAD PAIR:
        S^T[j, i] = sum_d kT[d, j] qT[d, i]           (two K=64 matmuls run
                                                       concurrently on PE row
                                                       groups 0-63 / 64-127)
        E = exp(S^T * scale)                          (ACT, fp16 out; no
                                                       max-subtract: |s*scale|
                                                       is small)
        racc[ih] += E                                 (DVE fp16 accumulate --
                                                       2-byte SBUF operands hit
                                                       the DVE fast path)
        O^T = V^T E                                   (COL-TILED: head A on PE
                                                       cols 0-63, head B on
                                                       cols 64-127; both K=128
                                                       streams run concurrently
                                                       and land on psum
                                                       partitions 0-63/64-127 --
                                                       exactly the channel-major
                                                       ot layout, no
                                                       partition-shuffling DMA)
        rsb = ones^T racc                             (col-tiled all-ones
                                                       reduce matmul: rowsum
                                                       lands BROADCAST on all
                                                       128 psum partitions,
                                                       aligned with O^T)
        ot = O^T * 1/rsb                              (partition-parallel DVE
                                                       reciprocal + one multiply
                                                       per ih -- no 1-lane ops)
  E:  yT[o, t] = sum_D WprojT[D, o] O^T[D, t] + bias[o]

Overlap structure: stage C is woven with pair 0's S^T/exp stream so ACT
starts early; each B tile-pair is emitted one head-pair ahead of the heads
that consume it; each pair's AV matmuls are woven one j-step behind its
S^T stream; the rowsum-reduce matmuls and normalization run after the next
pair's B matmuls so the racc tail latency never stalls the PE queue.

The ACT engine runs a pure exp stream (qkT psum->sbuf copies are on DVE);
softmax normalization needs no ACT work at all.

Host does only data movement: transposes / tiling rearranges of x and the
weights (cast to bf16), and the inverse transpose of the output.
"""

import json

import ml_dtypes
import numpy as np

D = 1024
NT = 1024
H = 16
HD = 64
P = 128
DC = D // P  # 8 contraction chunks
SCALE = HD ** -0.5
N_CORES = 8

_CACHE = {}


# --------------------------------------------------------------------------
# Workaround for the walrus build in this container: each TPB instruction
# encodes at most ONE sync wait (NEURON_ISA_TPB_EVENTS has a single wait
# slot) and this walrus version errors out instead of splitting. Tile
# attaches several waits per instruction. Hoist all but the last wait onto
# preceding single-wait EventSemaphore no-ops on the same (in-order) engine.
# --------------------------------------------------------------------------
def _split_sync_waits_json(bir_bytes: bytes) -> bytes:
    j = json.loads(bir_bytes)
    changed = False
    ctr = 0
    dma_ops = {"TensorLoad", "TensorSave", "TensorCopy", "TensorReduce"}
    for fn in j.get("functions", []):
        for blk in fn.get("blocks", []):
            out = []
            for inst in blk.get("instructions", []):
                si = inst.get("sync_info")
                if si:
                    waits = si.get("on_wait") or []
                    if len(waits) > 1:
                        for w in waits[:-1]:
                            ctr += 1
                            out.append(
                                {
                                    "debug": inst.get("debug", 0),
                                    "engine": inst.get("engine"),
                                    "ins": [],
                                    "outs": [],
                                    "name": f"splitw-{ctr}-{inst['name']}",
                                    "opcode": "EventSemaphore",
                                    "sync_info": {"on_update": [], "on_wait": [w]},
                                }
                            )
                        si["on_wait"] = [waits[-1]]
                        changed = True
                    ups = si.get("on_update") or []
                    if len(ups) > 1 and inst.get("opcode") not in dma_ops:
                        extra = ups[:-1]
                        si["on_update"] = [ups[-1]]
                        out.append(inst)
                        for u in extra:
                            ctr += 1
                            out.append(
                                {
                                    "debug": inst.get("debug", 0),
                                    "engine": inst.get("engine"),
                                    "ins": [],
                                    "outs": [],
                                    "name": f"splitu-{ctr}-{inst['name']}",
                                    "opcode": "EventSemaphore",
                                    "sync_info": {"on_update": [u], "on_wait": []},
                                }
                            )
                        changed = True
                        continue
                out.append(inst)
            blk["instructions"] = out
    if not changed:
        return bir_bytes
    return json.dumps(j).encode()


def _install_bir_fix():
    import concourse.bass as bass

    if getattr(bass.Bass, "_split_waits_patched", False):
        return
    orig = bass.Bass.to_json_bytes

    def patched(self, *a, **kw):
        return _split_sync_waits_json(orig(self, *a, **kw))

    bass.Bass.to_json_bytes = patched
    bass.Bass._split_waits_patched = True


def _build_module():
    from contextlib import ExitStack

    import concourse.bass as bass
    import concourse.tile as tile
    from concourse import mybir

    _install_bir_fix()
    f32 = mybir.dt.float32
    bf16 = mybir.dt.bfloat16
    f16 = mybir.dt.float16
    nc = bass.Bass(num_swdge_queues=4)

    xT = nc.declare_dram_parameter("xT", [D, NT], bf16, isOutput=False)
    # wqk[p, ct, a, c] = W_qkv.T[a*128+p, ct*128+c]  (q,k channels, ct<16)
    wqk = nc.declare_dram_parameter("wqk", [P, 16, DC, P], bf16, isOutput=False)
    # wv[p, a, cv] = W_qkv.T[a*128+p, 2048+cv]
    wvp = nc.declare_dram_parameter("wv", [P, DC, D], bf16, isOutput=False)
    # wpr[p, ot, a, c] = W_proj.T[a*128+p, ot*128+c]
    wpr = nc.declare_dram_parameter("wpr", [P, DC, DC, P], bf16, isOutput=False)
    # biasT[p, t] = b_proj[t*128+p]
    biasT = nc.declare_dram_parameter("biasT", [P, DC], f32, isOutput=False)
    yT = nc.declare_dram_parameter("yT", [D, NT], f32, isOutput=True)

    with tile.TileContext(nc) as tc, ExitStack() as outer:
        v_pool = outer.enter_context(tc.tile_pool(name="vsb", bufs=1))
        ot_pool = outer.enter_context(tc.tile_pool(name="otp", bufs=1))
        qk_pool = outer.enter_context(tc.tile_pool(name="qkp", bufs=4))
        misc = outer.enter_context(tc.tile_pool(name="misc", bufs=1))
        xt_pool = outer.enter_context(tc.tile_pool(name="xt", bufs=1))
        wt_pool = outer.enter_context(tc.tile_pool(name="wt", bufs=5))

        v_sb = v_pool.tile([P, DC, H, HD], f16)   # V, channel-major, fp16
        ot = ot_pool.tile([P, DC, NT], bf16)      # O^T, channel-major
        ones16 = misc.tile([P, HD], f16)          # all-ones rowsum stationary
        bias_t = misc.tile([P, DC], f32)
        nc.vector.memset(ones16[:], 1.0)
        nc.gpsimd.dma_start(bias_t[:], biasT[:])

        xt = xt_pool.tile([P, DC, NT], bf16)
        wt0 = wt_pool.tile([P, DC, P], bf16, tag="wt")
        nc.gpsimd.dma_start(wt0[:], wqk[:, 0, :, :])
        wt8 = wt_pool.tile([P, DC, P], bf16, tag="wt")
        nc.gpsimd.dma_start(wt8[:], wqk[:, 8, :, :])
        for a in range(DC):
            nc.gpsimd.dma_start(xt[:, a, :], xT[a * P : (a + 1) * P, :])

        # ------- stages B+D interleaved: qk projection + attention -------
        with (
            tc.tile_pool(name="es", bufs=18) as es_pool,
            tc.tile_pool(name="rac", bufs=4) as racc_pool,
            tc.tile_pool(name="rsp", bufs=4) as rs_pool,
            tc.tile_pool(name="psS", bufs=2, space="PSUM") as psS,
            tc.tile_pool(name="psO", bufs=1, space="PSUM") as psO,
            tc.tile_pool(name="psR", bufs=1, space="PSUM") as psR,
        ):

            def emit_b(ct, wt=None):
                # qkT[c, t] for one 128-channel tile (2 heads' q or k)
                if wt is None:
                    wt = wt_pool.tile([P, DC, P], bf16, tag="wt")
                    nc.gpsimd.dma_start(wt[:], wqk[:, ct, :, :])
                ps = psS.tile([P, NT], f32, tag="sps")
                for a in range(DC):
                    for nh in range(2):
                        nc.tensor.matmul(
                            ps[:, nh * 512 : (nh + 1) * 512],
                            wt[:, a, :],
                            xt[:, a, nh * 512 : (nh + 1) * 512],
                            start=(a == 0),
                            stop=(a == DC - 1),
                        )
                if ct < 8:
                    t = qk_pool.tile([P, NT], bf16, tag="qt")
                else:
                    t = qk_pool.tile([P, NT], bf16, tag="kt")
                nc.vector.tensor_copy(t[:], ps[:])
                return t

            # process heads in PAIRS: the two heads' K=64 S^T matmuls run
            # CONCURRENTLY on PE row groups 0-63 / 64-127 (row tiling), into
            # the two banks of a shared [P, 1024] PSUM tile, so one exp
            # covers both heads and the S^T wall halves.
            def pair_block(hp, qt, kt, es_pre=None, racc_pre=None):
                hA, hB = 2 * hp, 2 * hp + 1
                qsA, ksA = qt[0:HD, :], kt[0:HD, :]
                qsB, ksB = qt[HD:P, :], kt[HD:P, :]
                es_list = [None] * DC  # es_list[j] = (es_ih0, es_ih1)
                racc = [None, None]    # racc[ih]: running sum of es over j
                op = [None, None]      # op[ih]: O^T psum (A rows 0-63, B 64-127)

                def emit_st(j):
                    out = []
                    for ih in range(2):
                        sps = psS.tile([P, NT], f32, tag="sps")
                        for qs, ks, half in ((qsA, ksA, 0), (qsB, ksB, 1)):
                            nc.tensor.matmul(
                                sps[:, half * 512 : (half + 1) * 512],
                                ks[:, j * P : (j + 1) * P],
                                qs[:, ih * 512 : (ih + 1) * 512],
                                start=True,
                                stop=True,
                            )
                        es = es_pool.tile([P, NT], f16)
                        nc.scalar.activation(
                            es[:], sps[:], mybir.ActivationFunctionType.Exp,
                            scale=SCALE,
                        )
                        out.append(es)
                    return tuple(out)

                def acc_es(j):
                    # rowsum accumulate on DVE: fp16 SBUF operands stream at
                    # the 2-byte fast-path rate
                    for ih in range(2):
                        if j == 0:
                            racc[ih] = racc_pool.tile(
                                [P, NT], f16, tag=f"r{ih}", name=f"racc{ih}"
                            )
                            nc.vector.tensor_copy(racc[ih][:], es_list[0][ih][:])
                        else:
                            nc.vector.tensor_add(
                                racc[ih][:], racc[ih][:], es_list[j][ih][:]
                            )

                def do_av(j):
                    # col-tiled: head A on PE cols 0-63, head B on 64-127;
                    # the two moving streams run concurrently
                    for ih in range(2):
                        nc.tensor.matmul(
                            op[ih][0:HD, :],
                            v_sb[:, j, hA, :],
                            es_list[j][ih][:, 0:512],
                            start=(j == 0),
                            stop=(j == DC - 1),
                            tile_position=(0, 0),
                        )
                        nc.tensor.matmul(
                            op[ih][HD:P, :],
                            v_sb[:, j, hB, :],
                            es_list[j][ih][:, 512:1024],
                            start=(j == 0),
                            stop=(j == DC - 1),
                            tile_position=(0, HD),
                        )

                for j in range(DC):
                    if es_pre is not None:
                        es_list[j] = es_pre[j]
                    else:
                        es_list[j] = emit_st(j)
                    if racc_pre is not None:
                        racc[0], racc[1] = racc_pre
                    else:
                        acc_es(j)
                    if j >= 1:
                        if j == 1:
                            op[0] = psO.tile([P, 512], f32, tag="op0", name="op0")
                            op[1] = psO.tile([P, 512], f32, tag="op1", name="op1")
                        do_av(j - 1)
                do_av(DC - 1)
                return hp, op, racc

            def norm_pair(hp, op, racc):
                # rowsum-reduce (col-tiled all-ones matmul, broadcast over
                # all 128 out partitions), then partition-parallel
                # reciprocal + multiply. Runs after the NEXT pair's B
                # matmuls so the racc tail never stalls the PE queue.
                for ih in range(2):
                    rsb = psR.tile([P, 512], f32, tag=f"rsb{ih}")
                    nc.tensor.matmul(
                        rsb[0:HD, :],
                        ones16[:, 0:HD],
                        racc[ih][:, 0:512],
                        start=True, stop=True,
                        tile_position=(0, 0),
                    )
                    nc.tensor.matmul(
                        rsb[HD:P, :],
                        ones16[:, 0:HD],
                        racc[ih][:, 512:1024],
                        start=True, stop=True,
                        tile_position=(0, HD),
                    )
                    rsr = rs_pool.tile([P, 512], f32)
                    nc.vector.reciprocal(rsr[:], rsb[:])
                    nc.vector.tensor_mul(
                        ot[:, hp, ih * 512 : (ih + 1) * 512], op[ih][:], rsr[:]
                    )

            # ---- stage C (V = x @ Wv^T), woven with pair 0's S^T/exp ----
            qt = emit_b(0, wt=wt0)
            kt = emit_b(8, wt=wt8)
            es0 = [None] * DC
            racc0 = [None, None]
            with tc.tile_pool(name="wvt", bufs=1) as wv_pool:
                wv = wv_pool.tile([P, DC, D], bf16)
                nc.gpsimd.dma_start(wv[:], wvp[:])
                for vt in range(DC):
                    # alternate across the four attention psum tags so
                    # consecutive vt iterations double-buffer
                    if vt % 2 == 0:
                        pv0 = psO.tile([P, 512], f32, tag="op0")
                        pv1 = psO.tile([P, 512], f32, tag="op1")
                    else:
                        pv0 = psR.tile([P, 512], f32, tag="rsb0")
                        pv1 = psR.tile([P, 512], f32, tag="rsb1")
                    for a in range(DC):
                        for ch, ps in ((0, pv0), (1, pv1)):
                            nc.tensor.matmul(
                                ps[:],
                                xt[:, a, vt * P : (vt + 1) * P],
                                wv[:, a, ch * 512 : (ch + 1) * 512],
                                start=(a == 0),
                                stop=(a == DC - 1),
                            )
                    # weave pair 0's S^T so ACT starts its exps early
                    j = vt
                    for ih in range(2):
                        sps = psS.tile([P, NT], f32, tag="sps")
                        for qo2 in (0, HD):
                            nc.tensor.matmul(
                                sps[:, (qo2 // HD) * 512 : (qo2 // HD + 1) * 512],
                                kt[qo2 : qo2 + HD, j * P : (j + 1) * P],
                                qt[qo2 : qo2 + HD, ih * 512 : (ih + 1) * 512],
                                start=True,
                                stop=True,
                            )
                        es = es_pool.tile([P, NT], f16)
                        nc.scalar.activation(
                            es[:], sps[:], mybir.ActivationFunctionType.Exp,
                            scale=SCALE,
                        )
                        if es0[j] is None:
                            es0[j] = [None, None]
                        es0[j][ih] = es
                        if j == 0:
                            racc0[ih] = racc_pool.tile(
                                [P, NT], f16, tag=f"r{ih}", name=f"racc0{ih}"
                            )
                            nc.vector.tensor_copy(racc0[ih][:], es[:])
                        else:
                            nc.vector.tensor_add(racc0[ih][:], racc0[ih][:], es[:])
                    for ch, ps in ((0, pv0), (1, pv1)):
                        nc.vector.tensor_copy(
                            v_sb[:, vt, ch * 8 : (ch + 1) * 8, :],
                            ps[:].rearrange("p (h d) -> p h d", h=8),
                        )
            es0 = [tuple(e) for e in es0]

            for hp in range(8):
                res = pair_block(
                    hp, qt, kt,
                    es_pre=es0 if hp == 0 else None,
                    racc_pre=tuple(racc0) if hp == 0 else None,
                )
                if hp + 1 < 8:
                    qt = emit_b(hp + 1)
                    kt = emit_b(8 + hp + 1)
                norm_pair(*res)

        # -------- stage E: output projection + bias --------
        with (
            tc.tile_pool(name="wp", bufs=1) as wp_pool,
            tc.tile_pool(name="outp", bufs=3) as out_pool,
            tc.tile_pool(name="psE", bufs=2, space="PSUM") as psE,
        ):
            # prefetch the whole 2 MB of proj weights up front; the DMAs
            # overlap the tail of the attention phase
            wpt_all = wp_pool.tile([P, DC, DC, P], bf16)
            for oi in range(DC):
                nc.gpsimd.dma_start(wpt_all[:, oi, :, :], wpr[:, oi, :, :])
            for oi in range(DC):
                wpt = wpt_all[:, oi, :, :]
                osb = out_pool.tile([P, NT], f32)
                pe = psE.tile([P, NT], f32, tag="pse")
                for a in range(DC):
                    for nh in range(2):
                        nc.tensor.matmul(
                            pe[:, nh * 512 : (nh + 1) * 512],
                            wpt[:, a, :],
                            ot[:, a, nh * 512 : (nh + 1) * 512],
                            start=(a == 0),
                            stop=(a == DC - 1),
                        )
                nc.vector.tensor_scalar_add(osb[:], pe[:], bias_t[:, oi : oi + 1])
                nc.gpsimd.dma_start(yT[oi * P : (oi + 1) * P, :], osb[:])

    return nc


def _get_nc():
    if "nc" not in _CACHE:
        _CACHE["nc"] = _build_module()
    return _CACHE["nc"]


def _host_inputs(x, W_qkv, W_proj, b_proj):
    bf = ml_dtypes.bfloat16
    x = np.asarray(x, dtype=np.float32).astype(bf)
    W_qkv = np.asarray(W_qkv, dtype=np.float32).astype(bf)
    W_proj = np.asarray(W_proj, dtype=np.float32).astype(bf)
    b_proj = np.asarray(b_proj, dtype=np.float32)

    wqkvT = W_qkv.T  # [1024, 3072]
    # wqk[p, ct, a, c] = wqkvT[a*128+p, ct*128+c] for q,k channels
    wqk = np.ascontiguousarray(
        wqkvT[:, : 2 * D].reshape(DC, P, 16, P).transpose(1, 2, 0, 3)
    )
    # wv[p, a, cv] = wqkvT[a*128+p, 2048+cv]
    wv = np.ascontiguousarray(wqkvT[:, 2 * D :].reshape(DC, P, D).transpose(1, 0, 2))
    # wpr[p, ot, a, c] = W_proj.T[a*128+p, ot*128+c]
    wpr = np.ascontiguousarray(
        W_proj.T.reshape(DC, P, DC, P).transpose(1, 2, 0, 3)
    )
    biasT = np.ascontiguousarray(b_proj.reshape(DC, P).T)

    in_maps = []
    for i in range(N_CORES):
        in_maps.append(
            {
                "xT": np.ascontiguousarray(x[i].T),
                "wqk": wqk,
                "wv": wv,
                "wpr": wpr,
                "biasT": biasT,
            }
        )
    return in_maps


def _run(in_maps, trace=False):
    from concourse.bass_utils import run_bass_kernel_spmd

    nc = _get_nc()
    return run_bass_kernel_spmd(nc, in_maps, list(range(N_CORES)), trace=trace)


def kernel(x, W_qkv, W_proj, b_proj):
    in_maps = _host_inputs(x, W_qkv, W_proj, b_proj)
    res = _run(in_maps)
    out = np.stack([res.results[i]["yT"].T for i in range(N_CORES)], axis=0)
    return np.ascontiguousarray(out, dtype=np.float32)
